# revision 12
# baseline (speedup 1.0000x reference)
"""Causal self-attention with HoPE bias on 8 Trainium2 NeuronCores.

Sharding: core c handles batch b = c//2 and head group g = c%2 (4 of the 8
heads).  Each core computes q/k/v projections for its head slice, the causal
softmax attention for its 4 heads, and a partial output projection through
its slice of Wp.  The host gathers: out[b] = (partial_g0 + partial_g1).T
(bp is added on the g=0 core), v[b] is the concat of the two head groups.

Key device-side tricks:
- The HoPE bias is position-only (input-independent): active pairs cover
  global heads 0-3 only, and the position-independent remainder adds a
  per-head constant to all logits, which softmax cancels.  The Toeplitz bias
  sum_j cos((t-s)f_j)+sin((t-s)f_j) is expanded as a rank-64 inner product
  U(t).W(s), which is packed into the same K=128 matmul as q.k — the bias
  costs zero extra PE cycles.
- Scores are computed transposed ([s, t]) so the softmax denominator comes
  from an extra ones-column in the AV matmul, and the attention output lands
  in [head_dim, t] layout, which feeds the Wp matmul directly.
- All matmuls run in float32r (1 cycle/row vs 4 for fp32).
- The 1/sqrt(HD) softmax scale is folded into the Exp activation's scale
  with the bias tables pre-multiplied by 8.
"""

import os
import sys
import numpy as np
from contextlib import ExitStack

for _p in ("/opt/trn_rl_repo", "/root/.axon_site/_ro/trn_rl_repo"):
    if os.path.isdir(_p) and _p not in sys.path:
        sys.path.insert(0, _p)

import concourse.bacc as bacc
import concourse.mybir as mybir
import concourse.tile as tile
from concourse.bass_utils import run_bass_kernel_spmd

F32 = mybir.dt.float32
F32R = mybir.dt.float32r
AF = mybir.ActivationFunctionType
ALU = mybir.AluOpType

B, T, D, H = 4, 1024, 512, 8
HD = D // H          # 64
HG = 4               # heads per core
DG = HG * HD         # 256 = Dout slice per core
N_CORES = 8
SCALE = 0.125        # 1/sqrt(HD)


# ---------------------------------------------------------------- constants
def _hope_tables():
    """U[4,64,1024] (q-side, pre-scaled by 8) and W[4,64,1024] (k-side) per
    head group.  Group g=1 (global heads 4-7) is all zeros."""
    dim = D // 2                               # 256 pairs
    p = np.arange(dim)
    freqs = (10000.0 ** (-(p.astype(np.float64) / dim))).astype(np.float32)
    active = int((freqs.astype(np.float64) * 2.0 * np.pi * T >= 1.0).sum())
    active = min(active, dim - D // 4)         # 128
    t = np.arange(T, dtype=np.float64)
    tabs = {}
    for g in (0, 1):
        U = np.zeros((HG, HD, T), np.float32)
        W = np.zeros((HG, HD, T), np.float32)
        for hl in range(HG):
            gh = 4 * g + hl
            pairs = [j for j in range(active) if j // (HD // 2) == gh]
            for r, j in enumerate(pairs):
                f = float(freqs[j])
                c = np.cos(t * f)
                s = np.sin(t * f)
                U[hl, r] = 8.0 * c
                U[hl, 32 + r] = 8.0 * s
                W[hl, r] = (c - s).astype(np.float32)
                W[hl, 32 + r] = (s + c).astype(np.float32)
        tabs[g] = (U, W)
    return tabs


_HOPE = _hope_tables()


# ---------------------------------------------------------------- bass build
def _build_nc():
    nc = bacc.Bacc("TRN2", target_bir_lowering=False, debug=False,
                   num_devices=N_CORES)

    def din(name, shape):
        return nc.dram_tensor(name, shape, F32, kind="ExternalInput").ap()

    xin = din("xin", [T, D])
    v1s = din("v1s", [128, 8 * DG])          # [128, 2048] sc-major chunks
    wq = din("wq", [128, 4 * DG])            # [128, 1024] dc-major chunks
    wk = din("wk", [128, 4 * DG])
    wv = din("wv", [128, 4 * DG])
    wp = din("wp", [64, HG * D])             # [64, 2048] head-major chunks
    bq2 = din("bq2", [128, 2])
    bk2 = din("bk2", [128, 2])
    bvr = din("bvr", [128, DG])
    bpr = din("bpr", [128, 4])
    lam1 = din("lam1", [128, 1])
    lam2 = din("lam2", [128, 1])
    ucons = din("ucons", [HG, HD, T])
    wcons = din("wcons", [HG, HD, T])
    onesb = din("onesb", [128, 64])
    identin = din("identin", [128, 128])
    onesc = din("onesc", [128, 32])

    outp = nc.dram_tensor("outp", [D, T], F32, kind="ExternalOutput").ap()
    vout = nc.dram_tensor("vout", [T, DG], F32, kind="ExternalOutput").ap()

    with tile.TileContext(nc) as tc, ExitStack() as ctx:
        cst = ctx.enter_context(tc.tile_pool(name="cst", bufs=1))
        xload = ctx.enter_context(tc.tile_pool(name="xload", bufs=3))
        vfin = ctx.enter_context(tc.tile_pool(name="vfin", bufs=3))
        expp = ctx.enter_context(tc.tile_pool(name="expp", bufs=3))
        smal = ctx.enter_context(tc.tile_pool(name="smal", bufs=3))
        osb = ctx.enter_context(tc.tile_pool(name="osb", bufs=3))
        psA = ctx.enter_context(tc.tile_pool(name="psA", bufs=1, space="PSUM"))
        psB = ctx.enter_context(tc.tile_pool(name="psB", bufs=1, space="PSUM"))
        psC = ctx.enter_context(tc.tile_pool(name="psC", bufs=2, space="PSUM"))
        psD = ctx.enter_context(tc.tile_pool(name="psD", bufs=1, space="PSUM"))

        # persistent sbuf
        xT = cst.tile([128, 4 * T], F32R)         # [d-lane, dc*1024 + t]
        augQ = cst.tile([128, HG * T], F32R)      # per head: [kq|U] lanes
        augK = cst.tile([128, HG * T], F32R)
        v_aug = cst.tile([128, 8 * 260], F32R)    # per sc: 4x[v(64)|ones]
        attn = cst.tile([64, HG * T], F32R)       # per head: [hd, t]
        wq_t = cst.tile([128, 4 * DG], F32R)
        wk_t = cst.tile([128, 4 * DG], F32R)
        wv_t = cst.tile([128, 4 * DG], F32R)
        wp_t = cst.tile([64, HG * D], F32R)
        bq_t = cst.tile([128, 2], F32)
        bk_t = cst.tile([128, 2], F32)
        bvr_t = cst.tile([128, DG], F32)
        bpr_t = cst.tile([128, 4], F32)
        lam1_t = cst.tile([128, 1], F32)
        lam2_t = cst.tile([128, 1], F32)
        onesb_t = cst.tile([128, 64], F32R)
        ident = cst.tile([128, 128], F32R)
        lam1bv = cst.tile([128, DG], F32)

        nc.sync.dma_start(ident[:], identin[:].bitcast(F32R))

        # ---- phase 1: load x, transpose into xT ---------------------------
        for thalf in range(2):
            xt4 = [xload.tile([128, D], F32R, tag=f"xt{j}", name=f"xt{j}") for j in range(4)]
            for j in range(4):
                th = 4 * thalf + j
                nc.sync.dma_start(xt4[j][:], xin[128 * th:128 * (th + 1), :].bitcast(F32R))
            for dc in range(4):
                ps = psA.tile([128, 512], F32R, tag="big")
                for j in range(4):
                    nc.tensor.transpose(ps[:, 128 * j:128 * (j + 1)],
                                        xt4[j][:, 128 * dc:128 * (dc + 1)],
                                        ident[:])
                nc.vector.tensor_copy(
                    xT[:, T * dc + 512 * thalf:T * dc + 512 * (thalf + 1)],
                    ps[:])

        # ---- constants (weights on sync after x; tables on gpsimd) --------
        nc.sync.dma_start(wq_t[:], wq[:].bitcast(F32R))
        nc.sync.dma_start(wk_t[:], wk[:].bitcast(F32R))
        nc.sync.dma_start(wv_t[:], wv[:].bitcast(F32R))
        nc.sync.dma_start(wp_t[:], wp[:].bitcast(F32R))
        nc.sync.dma_start(bq_t[:], bq2[:])
        nc.sync.dma_start(bk_t[:], bk2[:])
        nc.sync.dma_start(bvr_t[:], bvr[:])
        nc.sync.dma_start(bpr_t[:], bpr[:])
        nc.sync.dma_start(lam1_t[:], lam1[:])
        nc.sync.dma_start(lam2_t[:], lam2[:])
        nc.sync.dma_start(onesb_t[:], onesb[:].bitcast(F32R))
        # hope tables into the const lanes of augQ/augK
        for h in range(HG):
            lo, hi = (64, 128) if h % 2 == 0 else (0, 64)
            nc.gpsimd.dma_start(augQ[lo:hi, T * h:T * (h + 1)],
                                ucons[h].bitcast(F32R))
            nc.gpsimd.dma_start(augK[lo:hi, T * h:T * (h + 1)],
                                wcons[h].bitcast(F32R))
        # ones columns of v_aug (col 65*k + 64 for k in 0..31)
        va_cols = v_aug[:].rearrange("p (s c) -> p s c", c=65)[:, :, 64:65]
        nc.gpsimd.dma_start(
            va_cols, onesc[:].rearrange("p (a b) -> p a b", b=1).bitcast(F32R))
        nc.vector.tensor_scalar_mul(lam1bv[:], bvr_t[:], lam1_t[:, 0:1])

        # ---- phase 2a: q/k projections into augQ/augK ---------------------
        for m in range(2):
            for tc2 in range(2):
                for (wt, bt, dstt, eng) in ((wq_t, bq_t, augQ, "q"),
                                            (wk_t, bk_t, augK, "k")):
                    ps = psB.tile([128, 512], F32, tag="proj")
                    for dc in range(4):
                        nc.tensor.matmul(
                            ps[:],
                            wt[:, DG * dc + 128 * m:DG * dc + 128 * (m + 1)],
                            xT[:, T * dc + 512 * tc2:T * dc + 512 * (tc2 + 1)],
                            start=(dc == 0), stop=(dc == 3))
                    for par in range(2):        # head 2m+par
                        h = 2 * m + par
                        lo, hi = (0, 64) if h % 2 == 0 else (64, 128)
                        dst = dstt[lo:hi,
                                   T * h + 512 * tc2:T * h + 512 * (tc2 + 1)]
                        if eng == "q":
                            nc.vector.tensor_scalar_add(
                                dst, ps[lo:hi, :], bt[lo:hi, m:m + 1])
                        else:
                            nc.scalar.activation(
                                dst, ps[lo:hi, :], AF.Identity,
                                bias=bt[lo:hi, m:m + 1])

        # ---- phase 2b: v projection, gating, v_aug ------------------------
        for sc in range(8):
            psv = psB.tile([128, DG], F32, tag="proj")
            for dc in range(4):
                nc.tensor.matmul(
                    psv[:],
                    xT[:, T * dc + 128 * sc:T * dc + 128 * (sc + 1)],
                    wv_t[:, DG * dc:DG * (dc + 1)],
                    start=(dc == 0), stop=(dc == 3))
            v1t = xload.tile([128, DG], F32, tag="v1t")
            nc.sync.dma_start(v1t[:], v1s[:, DG * sc:DG * (sc + 1)])
            ct = vfin.tile([128, DG], F32, tag="ct")
            # ct = lam2*v1 + lam1*bv
            nc.vector.scalar_tensor_tensor(
                ct[:], v1t[:], lam2_t[:, 0:1], lam1bv[:],
                op0=ALU.mult, op1=ALU.add)
            vf = vfin.tile([128, DG], F32, tag="vf")
            # vf = lam1*psv + ct
            nc.vector.scalar_tensor_tensor(
                vf[:], psv[:], lam1_t[:, 0:1], ct[:],
                op0=ALU.mult, op1=ALU.add)
            nc.gpsimd.dma_start(vout[128 * sc:128 * (sc + 1), :], vf[:])
            for h in range(HG):
                nc.vector.tensor_copy(
                    v_aug[:, 260 * sc + 65 * h:260 * sc + 65 * h + 64],
                    vf[:, 64 * h:64 * (h + 1)])

        # ---- phase 3: attention, (head, s-chunk)-major --------------------
        # For s-chunk i only t >= 128*i is needed (causal).  Scores for
        # (h, i) are computed into a [128, 1024-128i] psum (split into <=512
        # matmuls), exp'd in one ACT pass, diagonal-masked on the first 128
        # cols only, then accumulated into the per-(h, J) AV psums.
        for h in range(HG):
            av_ps = {J: psD.tile([128, 512], F32, tag=f"av{J}",
                                 name=f"avp{h}{J}") for J in range(2)}
            for i in range(8):
                tlo = 128 * i              # first valid t col
                ncols = T - tlo
                sp = psC.tile([128, 1024], F32, tag="sc", name=f"sp{h}{i}")
                for part in range(2 if ncols > 512 else 1):
                    c0 = tlo + 512 * part
                    c1 = min(c0 + 512, T)
                    nc.tensor.matmul(
                        sp[:, c0 - tlo:c1 - tlo],
                        augK[:, T * h + 128 * i:T * h + 128 * (i + 1)],
                        augQ[:, T * h + c0:T * h + c1],
                        start=True, stop=True)
                et = expp.tile([128, 1024], F32R, tag="et", name=f"et{h}{i}")
                nc.scalar.activation(et[:, 0:ncols], sp[:, 0:ncols],
                                     AF.Exp, scale=SCALE)
                # mask the diagonal 128 cols: keep where t_l - s_l >= 0
                nc.gpsimd.affine_select(
                    out=et[:, 0:128], in_=et[:, 0:128],
                    compare_op=ALU.is_ge, fill=0.0, base=0,
                    pattern=[[1, 128]], channel_multiplier=-1)
                for J in range(2):
                    jlo, jhi = 512 * J, 512 * (J + 1)
                    if jhi <= tlo:
                        continue
                    e0 = max(jlo, tlo) - tlo
                    e1 = jhi - tlo
                    nc.tensor.matmul(
                        av_ps[J][0:65, max(jlo, tlo) - jlo:512],
                        v_aug[:, 260 * i + 65 * h:260 * i + 65 * (h + 1)],
                        et[:, e0:e1],
                        start=(i == 0), stop=(i == (3 if J == 0 else 7)))
            for J in range(2):
                avp = av_ps[J]
                # denominator -> broadcast -> approx reciprocal -> multiply
                den = smal.tile([128, 512], F32R, tag="den", name=f"den{h}{J}")
                nc.vector.tensor_copy(den[64:65, :], avp[64:65, :])
                bc = psB.tile([64, 512], F32, tag="proj", name=f"bc{h}{J}")
                nc.tensor.matmul(bc[:], onesb_t[64:65, :], den[64:65, :],
                                 start=True, stop=True)
                rec = smal.tile([64, 512], F32, tag="rec", name=f"rec{h}{J}")
                nc.vector.reciprocal_approx_fast(rec[:], bc[:])
                nc.vector.tensor_mul(
                    attn[0:64, T * h + 512 * J:T * h + 512 * (J + 1)],
                    avp[0:64, :], rec[:])

        # ---- phase 4: partial output projection ---------------------------
        for J in range(2):
            for od in range(4):
                op = psA.tile([128, 512], F32, tag="big", name=f"op{J}{od}")
                for h in range(HG):
                    nc.tensor.matmul(
                        op[:],
                        wp_t[0:64, D * h + 128 * od:D * h + 128 * (od + 1)],
                        attn[0:64, T * h + 512 * J:T * h + 512 * (J + 1)],
                        start=(h == 0), stop=(h == HG - 1))
                ot = osb.tile([128, 512], F32, tag="ot", name=f"ot{J}{od}")
                nc.scalar.activation(ot[:], op[:], AF.Identity,
                                     bias=bpr_t[:, od:od + 1])
                nc.sync.dma_start(
                    outp[128 * od:128 * (od + 1), 512 * J:512 * (J + 1)],
                    ot[:])

    nc.compile()
    return nc


_NC_CACHE = None


def _get_nc():
    global _NC_CACHE
    if _NC_CACHE is None:
        _NC_CACHE = _build_nc()
    return _NC_CACHE


# ---------------------------------------------------------------- host side
def _core_inputs(c, x, v1, Wq, bq, Wk, bk, Wv, bv, Wp, bp, lamb1, lamb2):
    b, g = c // 2, c % 2
    sl = slice(DG * g, DG * (g + 1))
    U, W = _HOPE[g]

    def chunk_cols(w):  # [512, 256] -> [128, 4*256] dc-major
        return np.ascontiguousarray(
            w.reshape(4, 128, DG).transpose(1, 0, 2).reshape(128, 4 * DG))

    wp_l = np.ascontiguousarray(
        Wp[sl, :].reshape(HG, 64, D).transpose(1, 0, 2).reshape(64, HG * D))
    bpr = (bp.reshape(4, 128).T.copy() if g == 0
           else np.zeros((128, 4), np.float32))
    v1b = np.ascontiguousarray(
        v1[b, :, 4 * g:4 * (g + 1), :].reshape(8, 128, DG)
        .transpose(1, 0, 2).reshape(128, 8 * DG))
    return {
        "xin": np.ascontiguousarray(x[b]),
        "v1s": v1b,
        "wq": chunk_cols(Wq[:, sl]),
        "wk": chunk_cols(Wk[:, sl]),
        "wv": chunk_cols(Wv[:, sl]),
        "wp": wp_l,
        "bq2": np.ascontiguousarray(bq[sl].reshape(2, 128).T),
        "bk2": np.ascontiguousarray(bk[sl].reshape(2, 128).T),
        "bvr": np.broadcast_to(bv[sl], (128, DG)).copy(),
        "bpr": np.ascontiguousarray(bpr.astype(np.float32)),
        "lam1": np.full((128, 1), np.float32(lamb1), np.float32),
        "lam2": np.full((128, 1), np.float32(lamb2), np.float32),
        "ucons": U,
        "wcons": W,
        "onesb": np.ones((128, 64), np.float32),
        "identin": np.eye(128, dtype=np.float32),
        "onesc": np.ones((128, 32), np.float32),
    }


def kernel(x, v1, Wq, bq, Wk, bk, Wv, bv, Wp, bp, lamb1, lamb2,
           pos_independent):
    x = np.asarray(x, np.float32)
    v1 = np.asarray(v1, np.float32)
    args = [np.asarray(a, np.float32) for a in
            (Wq, bq, Wk, bk, Wv, bv, Wp, bp)]
    nc = _get_nc()
    in_maps = [_core_inputs(c, x, v1, *args, np.float32(lamb1),
                            np.float32(lamb2)) for c in range(N_CORES)]
    trace = bool(int(os.environ.get("KERNEL_TRACE", "0")))
    res = run_bass_kernel_spmd(nc, in_maps, core_ids=list(range(N_CORES)),
                               trace=trace)
    kernel.last_results = res
    out = np.empty((B, T, D), np.float32)
    v = np.empty((B, T, H, HD), np.float32)
    for b in range(B):
        r0, r1 = res.results[2 * b], res.results[2 * b + 1]
        out[b] = (r0["outp"] + r1["outp"]).T
        v[b, :, 0:4, :] = r0["vout"].reshape(T, HG, HD)
        v[b, :, 4:8, :] = r1["vout"].reshape(T, HG, HD)
    return out, v


# revision 13
# speedup vs baseline: 1.2630x; 1.2630x over previous
"""Causal self-attention with HoPE bias on 8 Trainium2 NeuronCores.

Sharding: core c handles batch b = c//2 and head group g = c%2 (4 of the 8
heads).  Each core computes q/k/v projections for its head slice, the causal
softmax attention for its 4 heads, and a partial output projection through
its slice of Wp.  The host gathers: out[b] = (partial_g0 + partial_g1).T
(bp is added on the g=0 core), v[b] is the concat of the two head groups.

Key device-side tricks:
- The HoPE bias is position-only (input-independent): active pairs cover
  global heads 0-3 only, and the position-independent remainder adds a
  per-head constant to all logits, which softmax cancels.  The Toeplitz bias
  sum_j cos((t-s)f_j)+sin((t-s)f_j) is expanded as a rank-64 inner product
  U(t).W(s), which is packed into the same K=128 matmul as q.k — the bias
  costs zero extra PE cycles.
- Scores are computed transposed ([s, t]) so the softmax denominator comes
  from an extra ones-column in the AV matmul, and the attention output lands
  in [head_dim, t] layout, which feeds the Wp matmul directly.
- All matmuls run in float32r (1 cycle/row vs 4 for fp32).
- The 1/sqrt(HD) softmax scale is folded into the Exp activation's scale
  with the bias tables pre-multiplied by 8.
"""

import os
import sys
import numpy as np
from contextlib import ExitStack

for _p in ("/opt/trn_rl_repo", "/root/.axon_site/_ro/trn_rl_repo"):
    if os.path.isdir(_p) and _p not in sys.path:
        sys.path.insert(0, _p)

import concourse.bacc as bacc
import concourse.mybir as mybir
import concourse.tile as tile
from concourse.bass_utils import run_bass_kernel_spmd

F32 = mybir.dt.float32
F32R = mybir.dt.float32r
AF = mybir.ActivationFunctionType
ALU = mybir.AluOpType

B, T, D, H = 4, 1024, 512, 8
HD = D // H          # 64
HG = 4               # heads per core
DG = HG * HD         # 256 = Dout slice per core
N_CORES = 8
SCALE = 0.125        # 1/sqrt(HD)


# ---------------------------------------------------------------- constants
def _hope_tables():
    """U[4,64,1024] (q-side, pre-scaled by 8) and W[4,64,1024] (k-side) per
    head group.  Group g=1 (global heads 4-7) is all zeros."""
    dim = D // 2                               # 256 pairs
    p = np.arange(dim)
    freqs = (10000.0 ** (-(p.astype(np.float64) / dim))).astype(np.float32)
    active = int((freqs.astype(np.float64) * 2.0 * np.pi * T >= 1.0).sum())
    active = min(active, dim - D // 4)         # 128
    t = np.arange(T, dtype=np.float64)
    tabs = {}
    for g in (0, 1):
        U = np.zeros((HG, HD, T), np.float32)
        W = np.zeros((HG, HD, T), np.float32)
        for hl in range(HG):
            gh = 4 * g + hl
            pairs = [j for j in range(active) if j // (HD // 2) == gh]
            for r, j in enumerate(pairs):
                f = float(freqs[j])
                c = np.cos(t * f)
                s = np.sin(t * f)
                U[hl, r] = 8.0 * c
                U[hl, 32 + r] = 8.0 * s
                W[hl, r] = (c - s).astype(np.float32)
                W[hl, 32 + r] = (s + c).astype(np.float32)
        tabs[g] = (U, W)
    return tabs


_HOPE = _hope_tables()


# ---------------------------------------------------------------- bass build
def _build_nc():
    nc = bacc.Bacc("TRN2", target_bir_lowering=False, debug=False,
                   num_devices=N_CORES)

    def din(name, shape):
        return nc.dram_tensor(name, shape, F32, kind="ExternalInput").ap()

    xin = din("xin", [T, D])
    v1s = din("v1s", [128, 8 * DG])          # [128, 2048] sc-major chunks
    wq = din("wq", [128, 4 * DG])            # [128, 1024] dc-major chunks
    wk = din("wk", [128, 4 * DG])
    wv = din("wv", [128, 4 * DG])
    wp = din("wp", [64, HG * D])             # [64, 2048] head-major chunks
    bq2 = din("bq2", [128, 2])
    bk2 = din("bk2", [128, 2])
    bvr = din("bvr", [128, DG])
    bpr = din("bpr", [128, 4])
    lam1 = din("lam1", [128, 1])
    lam2 = din("lam2", [128, 1])
    ucons = din("ucons", [HG, HD, T])
    wcons = din("wcons", [HG, HD, T])
    onesb = din("onesb", [128, 64])
    identin = din("identin", [128, 128])
    onesc = din("onesc", [128, 32])

    outp = nc.dram_tensor("outp", [D, T], F32, kind="ExternalOutput").ap()
    vout = nc.dram_tensor("vout", [T, DG], F32, kind="ExternalOutput").ap()

    with tile.TileContext(nc) as tc, ExitStack() as ctx:
        cst = ctx.enter_context(tc.tile_pool(name="cst", bufs=1))
        xload = ctx.enter_context(tc.tile_pool(name="xload", bufs=3))
        vfin = ctx.enter_context(tc.tile_pool(name="vfin", bufs=3))
        expp = ctx.enter_context(tc.tile_pool(name="expp", bufs=3))
        smal = ctx.enter_context(tc.tile_pool(name="smal", bufs=3))
        osb = ctx.enter_context(tc.tile_pool(name="osb", bufs=3))
        psA = ctx.enter_context(tc.tile_pool(name="psA", bufs=2, space="PSUM"))
        psB = ctx.enter_context(tc.tile_pool(name="psB", bufs=2, space="PSUM"))
        psC = ctx.enter_context(tc.tile_pool(name="psC", bufs=2, space="PSUM"))
        psD = ctx.enter_context(tc.tile_pool(name="psD", bufs=1, space="PSUM"))

        # persistent sbuf
        xT = cst.tile([128, 4 * T], F32R)         # [d-lane, dc*1024 + t]
        augQ = cst.tile([128, HG * T], F32R)      # per head: [kq|U] lanes
        augK = cst.tile([128, HG * T], F32R)
        v_aug = cst.tile([128, 8 * 260], F32R)    # per sc: 4x[v(64)|ones]
        attn = cst.tile([64, HG * T], F32R)       # per head: [hd, t]
        wq_t = cst.tile([128, 4 * DG], F32R)
        wk_t = cst.tile([128, 4 * DG], F32R)
        wv_t = cst.tile([128, 4 * DG], F32R)
        wp_t = cst.tile([64, HG * D], F32R)
        bq_t = cst.tile([128, 2], F32)
        bk_t = cst.tile([128, 2], F32)
        bvr_t = cst.tile([128, DG], F32)
        bpr_t = cst.tile([128, 4], F32)
        lam1_t = cst.tile([128, 1], F32)
        lam2_t = cst.tile([128, 1], F32)
        onesb_t = cst.tile([128, 64], F32R)
        ident = cst.tile([128, 128], F32R)
        lam1bv = cst.tile([128, DG], F32)

        nc.sync.dma_start(ident[:], identin[:].bitcast(F32R))

        # ---- phase 1: load x, transpose into xT ---------------------------
        for thalf in range(2):
            xt4 = [xload.tile([128, D], F32R, tag=f"xt{j}", name=f"xt{j}") for j in range(4)]
            for j in range(4):
                th = 4 * thalf + j
                nc.sync.dma_start(xt4[j][:], xin[128 * th:128 * (th + 1), :].bitcast(F32R))
            for dc in range(4):
                ps = psA.tile([128, 512], F32R, tag="big")
                for j in range(4):
                    nc.tensor.transpose(ps[:, 128 * j:128 * (j + 1)],
                                        xt4[j][:, 128 * dc:128 * (dc + 1)],
                                        ident[:])
                nc.vector.tensor_copy(
                    xT[:, T * dc + 512 * thalf:T * dc + 512 * (thalf + 1)],
                    ps[:])

        # ---- constants (weights on sync after x; tables on gpsimd) --------
        nc.sync.dma_start(wq_t[:], wq[:].bitcast(F32R))
        nc.sync.dma_start(wk_t[:], wk[:].bitcast(F32R))
        nc.sync.dma_start(wv_t[:], wv[:].bitcast(F32R))
        nc.sync.dma_start(wp_t[:], wp[:].bitcast(F32R))
        nc.sync.dma_start(bq_t[:], bq2[:])
        nc.sync.dma_start(bk_t[:], bk2[:])
        nc.sync.dma_start(bvr_t[:], bvr[:])
        nc.sync.dma_start(bpr_t[:], bpr[:])
        nc.sync.dma_start(lam1_t[:], lam1[:])
        nc.sync.dma_start(lam2_t[:], lam2[:])
        nc.sync.dma_start(onesb_t[:], onesb[:].bitcast(F32R))
        # hope tables into the const lanes of augQ/augK
        for h in range(HG):
            lo, hi = (64, 128) if h % 2 == 0 else (0, 64)
            nc.gpsimd.dma_start(augQ[lo:hi, T * h:T * (h + 1)],
                                ucons[h].bitcast(F32R))
            nc.gpsimd.dma_start(augK[lo:hi, T * h:T * (h + 1)],
                                wcons[h].bitcast(F32R))
        # ones columns of v_aug (col 65*k + 64 for k in 0..31)
        va_cols = v_aug[:].rearrange("p (s c) -> p s c", c=65)[:, :, 64:65]
        nc.gpsimd.dma_start(
            va_cols, onesc[:].rearrange("p (a b) -> p a b", b=1).bitcast(F32R))
        nc.vector.tensor_scalar_mul(lam1bv[:], bvr_t[:], lam1_t[:, 0:1])

        # ---- phase 2a: q/k projections into augQ/augK ---------------------
        for m in range(2):
            for tc2 in range(2):
                for (wt, bt, dstt, eng) in ((wq_t, bq_t, augQ, "q"),
                                            (wk_t, bk_t, augK, "k")):
                    ps = psB.tile([128, 512], F32, tag="proj")
                    for dc in range(4):
                        nc.tensor.matmul(
                            ps[:],
                            wt[:, DG * dc + 128 * m:DG * dc + 128 * (m + 1)],
                            xT[:, T * dc + 512 * tc2:T * dc + 512 * (tc2 + 1)],
                            start=(dc == 0), stop=(dc == 3))
                    for par in range(2):        # head 2m+par
                        h = 2 * m + par
                        lo, hi = (0, 64) if h % 2 == 0 else (64, 128)
                        dst = dstt[lo:hi,
                                   T * h + 512 * tc2:T * h + 512 * (tc2 + 1)]
                        if eng == "q":
                            nc.vector.tensor_scalar_add(
                                dst, ps[lo:hi, :], bt[lo:hi, m:m + 1])
                        else:
                            nc.scalar.activation(
                                dst, ps[lo:hi, :], AF.Identity,
                                bias=bt[lo:hi, m:m + 1])

        # ---- phase 2b: v projection, gating, v_aug ------------------------
        for sc in range(8):
            psv = psB.tile([128, DG], F32, tag="proj")
            for dc in range(4):
                nc.tensor.matmul(
                    psv[:],
                    xT[:, T * dc + 128 * sc:T * dc + 128 * (sc + 1)],
                    wv_t[:, DG * dc:DG * (dc + 1)],
                    start=(dc == 0), stop=(dc == 3))
            v1t = xload.tile([128, DG], F32, tag="v1t")
            nc.sync.dma_start(v1t[:], v1s[:, DG * sc:DG * (sc + 1)])
            ct = vfin.tile([128, DG], F32, tag="ct")
            # ct = lam2*v1 + lam1*bv
            nc.vector.scalar_tensor_tensor(
                ct[:], v1t[:], lam2_t[:, 0:1], lam1bv[:],
                op0=ALU.mult, op1=ALU.add)
            vf = vfin.tile([128, DG], F32, tag="vf")
            # vf = lam1*psv + ct
            nc.vector.scalar_tensor_tensor(
                vf[:], psv[:], lam1_t[:, 0:1], ct[:],
                op0=ALU.mult, op1=ALU.add)
            nc.sync.dma_start(vout[128 * sc:128 * (sc + 1), :], vf[:])
            for h in range(HG):
                nc.vector.tensor_copy(
                    v_aug[:, 260 * sc + 65 * h:260 * sc + 65 * h + 64],
                    vf[:, 64 * h:64 * (h + 1)])

        # ---- phase 3: attention -------------------------------------------
        # scoresT[s,t] per (head, J-chunk, s-chunk i): only t >= 128*i is
        # computed (causal).  The diagonal mask applies to the first 128
        # cols of the i==4J..4J+3 tiles only.
        for h in range(HG):
            av_ps = {J: psD.tile([128, 512], F32, tag=f"av{J}", bufs=1,
                                 name=f"avp{h}{J}") for J in range(2)}
            for J in range(2):
                for i in range(4 * (J + 1)):
                    tlo = max(512 * J, 128 * i)   # first valid t col
                    ncols = 512 * (J + 1) - tlo
                    sp = psC.tile([128, 512], F32, tag="sc", name=f"sp{h}{J}{i}")
                    nc.tensor.matmul(
                        sp[:, 0:ncols],
                        augK[:, T * h + 128 * i:T * h + 128 * (i + 1)],
                        augQ[:, T * h + tlo:T * h + tlo + ncols],
                        start=True, stop=True)
                    et = expp.tile([128, 512], F32R, tag="et",
                                   name=f"et{h}{J}{i}")
                    nc.scalar.activation(et[:, 0:ncols], sp[:, 0:ncols],
                                         AF.Exp, scale=SCALE)
                    if i >= 4 * J:
                        # diagonal 128 cols: keep where t_l - s_l >= 0
                        nc.gpsimd.affine_select(
                            out=et[:, 0:128], in_=et[:, 0:128],
                            compare_op=ALU.is_ge, fill=0.0, base=0,
                            pattern=[[1, 128]], channel_multiplier=-1)
                    nc.tensor.matmul(
                        av_ps[J][0:65, tlo - 512 * J:512],
                        v_aug[:, 260 * i + 65 * h:260 * i + 65 * (h + 1)],
                        et[:, 0:ncols],
                        start=(i == 0), stop=(i == 4 * J + 3))
            for J in range(2):
                avp = av_ps[J]
                # denominator -> broadcast -> approx reciprocal -> multiply
                den = smal.tile([128, 512], F32R, tag="den", name=f"den{h}{J}")
                nc.vector.tensor_copy(den[64:65, :], avp[64:65, :])
                bc = psB.tile([64, 512], F32, tag="proj", name=f"bc{h}{J}")
                nc.tensor.matmul(bc[:], onesb_t[64:65, :], den[64:65, :],
                                 start=True, stop=True)
                rec = smal.tile([64, 512], F32, tag="rec", name=f"rec{h}{J}")
                nc.vector.reciprocal_approx_fast(rec[:], bc[:])
                nc.vector.tensor_mul(
                    attn[0:64, T * h + 512 * J:T * h + 512 * (J + 1)],
                    avp[0:64, :], rec[:])

        # ---- phase 4: partial output projection ---------------------------
        for J in range(2):
            for od in range(4):
                op = psA.tile([128, 512], F32, tag="big", name=f"op{J}{od}")
                for h in range(HG):
                    nc.tensor.matmul(
                        op[:],
                        wp_t[0:64, D * h + 128 * od:D * h + 128 * (od + 1)],
                        attn[0:64, T * h + 512 * J:T * h + 512 * (J + 1)],
                        start=(h == 0), stop=(h == HG - 1))
                ot = osb.tile([128, 512], F32, tag="ot", name=f"ot{J}{od}")
                nc.scalar.activation(ot[:], op[:], AF.Identity,
                                     bias=bpr_t[:, od:od + 1])
                nc.sync.dma_start(
                    outp[128 * od:128 * (od + 1), 512 * J:512 * (J + 1)],
                    ot[:])

    nc.compile()
    return nc


_NC_CACHE = None


def _get_nc():
    global _NC_CACHE
    if _NC_CACHE is None:
        _NC_CACHE = _build_nc()
    return _NC_CACHE


# ---------------------------------------------------------------- host side
def _core_inputs(c, x, v1, Wq, bq, Wk, bk, Wv, bv, Wp, bp, lamb1, lamb2):
    b, g = c // 2, c % 2
    sl = slice(DG * g, DG * (g + 1))
    U, W = _HOPE[g]

    def chunk_cols(w):  # [512, 256] -> [128, 4*256] dc-major
        return np.ascontiguousarray(
            w.reshape(4, 128, DG).transpose(1, 0, 2).reshape(128, 4 * DG))

    wp_l = np.ascontiguousarray(
        Wp[sl, :].reshape(HG, 64, D).transpose(1, 0, 2).reshape(64, HG * D))
    bpr = (bp.reshape(4, 128).T.copy() if g == 0
           else np.zeros((128, 4), np.float32))
    v1b = np.ascontiguousarray(
        v1[b, :, 4 * g:4 * (g + 1), :].reshape(8, 128, DG)
        .transpose(1, 0, 2).reshape(128, 8 * DG))
    return {
        "xin": np.ascontiguousarray(x[b]),
        "v1s": v1b,
        "wq": chunk_cols(Wq[:, sl]),
        "wk": chunk_cols(Wk[:, sl]),
        "wv": chunk_cols(Wv[:, sl]),
        "wp": wp_l,
        "bq2": np.ascontiguousarray(bq[sl].reshape(2, 128).T),
        "bk2": np.ascontiguousarray(bk[sl].reshape(2, 128).T),
        "bvr": np.broadcast_to(bv[sl], (128, DG)).copy(),
        "bpr": np.ascontiguousarray(bpr.astype(np.float32)),
        "lam1": np.full((128, 1), np.float32(lamb1), np.float32),
        "lam2": np.full((128, 1), np.float32(lamb2), np.float32),
        "ucons": U,
        "wcons": W,
        "onesb": np.ones((128, 64), np.float32),
        "identin": np.eye(128, dtype=np.float32),
        "onesc": np.ones((128, 32), np.float32),
    }


def kernel(x, v1, Wq, bq, Wk, bk, Wv, bv, Wp, bp, lamb1, lamb2,
           pos_independent):
    x = np.asarray(x, np.float32)
    v1 = np.asarray(v1, np.float32)
    args = [np.asarray(a, np.float32) for a in
            (Wq, bq, Wk, bk, Wv, bv, Wp, bp)]
    nc = _get_nc()
    in_maps = [_core_inputs(c, x, v1, *args, np.float32(lamb1),
                            np.float32(lamb2)) for c in range(N_CORES)]
    trace = bool(int(os.environ.get("KERNEL_TRACE", "0")))
    res = run_bass_kernel_spmd(nc, in_maps, core_ids=list(range(N_CORES)),
                               trace=trace)
    kernel.last_results = res
    out = np.empty((B, T, D), np.float32)
    v = np.empty((B, T, H, HD), np.float32)
    for b in range(B):
        r0, r1 = res.results[2 * b], res.results[2 * b + 1]
        out[b] = (r0["outp"] + r1["outp"]).T
        v[b, :, 0:4, :] = r0["vout"].reshape(T, HG, HD)
        v[b, :, 4:8, :] = r1["vout"].reshape(T, HG, HD)
    return out, v
